# revision 22
# baseline (speedup 1.0000x reference)
"""Bahdanau attention kernel for Trainium2 (8 NeuronCores, SPMD batch-parallel).

Problem: B=32, T=8192, D=256, U=128
  h_v   = relu(values @ W1 + b1)            [B, T, U]
  h_q   = relu(query  @ W2 + b2)            [B, U]
  score = tanh(h_v + h_q) @ V + Vb          [B, T, 1]
  attn  = softmax(score, axis=T)            [B, T, 1]
  ctx   = sum(attn * values, axis=T)        [B, D]

Sharding: batch B=32 split across 8 cores (4 batches each); weights replicated.

Per-core layout strategy (the PE contracts over the partition dim, so the two
big matmuls need opposite orientations of `values`):
  - W1 matmul consumes values^T tiles  [d on partitions, t free]
  - context matmul consumes row tiles  [t on partitions, d free]
The host stages BOTH layouts in bf16 (4MB + 4MB per batch) -- the same HBM
byte count as one fp32 copy, so the memory roofline is unchanged and no
on-chip transpose/cast pass is needed.

T-order permutation: the kernel processes "pipeline positions" i = c*128 + p
(score chunk c, partition p) which the host maps to true t = p*64 + c.  With
this permutation the natural-layout load gives each partition one contiguous
32KB HBM run, and the attention-weight store is per-partition contiguous --
no on-chip transpose and near-ideal DMA descriptors in both directions.

Softmax is computed without max-subtraction (|score| <= sum|V| ~ 5, exp is
safe in fp32) and normalization is folded to the end:
  e = exp(score); attn = e/Z; ctx = (sum_t e_t v_t)/Z.
V_bias shifts every score by a constant and cancels in softmax, so it cannot
influence either output.
"""

import os
import numpy as np
import ml_dtypes

# fp8 staging of the transposed values copy (the W1/score path only; the
# context path stays bf16).  W1 is scaled by 16 before the fp8 cast and the
# scale is exactly compensated in the tanh's free affine (scale=1/16).
VT_FP8 = os.environ.get("VT_FP8", "0") == "1"

B, T, D, U = 32, 8192, 256, 128
NCORES = 8
NB = B // NCORES        # batches per core
TILE = 512              # t-tile for the W1 matmul / activations
NT = T // TILE          # 16
NCH = T // 128          # 64 score/context chunks
CPP = T // 128 // 2     # 64 columns per partition in attn layout (= NCH)

BF16 = ml_dtypes.bfloat16

_compiled = None


def _build_bass(repeat=1):
    import concourse.bass as bass
    import concourse.tile as tile
    from concourse import bacc, mybir
    from concourse.mybir import AluOpType as alu
    from concourse.mybir import ActivationFunctionType as act

    f32 = mybir.dt.float32
    bf16 = mybir.dt.bfloat16
    vdt = mybir.dt.float8e4 if VT_FP8 else bf16
    ts = bass.ts

    nc = bacc.Bacc(
        "TRN2",
        target_bir_lowering=False,
        debug=False,
        enable_asserts=False,
        num_devices=NCORES,
    )

    # ---- DRAM I/O ----
    # vT: values transposed+permuted  [NB, 2, 128, T] laid as (k, p) x i
    vT = nc.dram_tensor("vT", [NB, 2, 128, T], vdt, kind="ExternalInput").ap()
    # vn: values rows in permuted order: row (p, c) = values[p*64 + c]
    vn = nc.dram_tensor("vn", [NB, 128, NCH, D], bf16, kind="ExternalInput").ap()
    # misc f32 consts packed [128, 2 + 2*NB + 1]: b1 | b2 | qT(2*NB) | ones
    miscf = nc.dram_tensor("miscf", [128, 3 + 2 * NB], f32,
                           kind="ExternalInput").ap()
    # w2 f32 [128, 2, U]
    w2 = nc.dram_tensor("w2", [128, 2, U], f32, kind="ExternalInput").ap()
    # w1 in the vT dtype (scaled x16 by the host when fp8)
    w1t = nc.dram_tensor("w1t", [128, 2, U], vdt, kind="ExternalInput").ap()
    # bf16 consts [128, 1]: vk
    miscb = nc.dram_tensor("miscb", [128, 1], bf16, kind="ExternalInput").ap()
    ctx_out = nc.dram_tensor("ctx_out", [NB, D], f32, kind="ExternalOutput").ap()
    attn_out = nc.dram_tensor("attn_out", [NB, T], f32, kind="ExternalOutput").ap()

    # attn rows: partition p holds t in [p*64, p*64+64)
    attn_r = attn_out.rearrange("nb (p c) -> p nb c", p=128)   # [128, NB, NCH]

    with tile.TileContext(nc) as tc:
        with (
            tc.tile_pool(name="const", bufs=1) as const,
            tc.tile_pool(name="vts", bufs=2) as vts_pool,
            tc.tile_pool(name="vns", bufs=2) as vns_pool,
            tc.tile_pool(name="zp", bufs=3) as zpool,
            tc.tile_pool(name="ztp", bufs=3) as ztpool,
            tc.tile_pool(name="ep", bufs=2) as epool,
            tc.tile_pool(name="op", bufs=2) as outpool,
            tc.tile_pool(name="ph", bufs=2, space="PSUM") as ph_pool,
            tc.tile_pool(name="scp", bufs=2, space="PSUM") as sc_pool,
            tc.tile_pool(name="cxp", bufs=2, space="PSUM") as cx_pool,
            tc.tile_pool(name="psm", bufs=1, space="PSUM") as psmall,
        ):
            # ---- constants (4 small DMAs, on the ACT ring so the big
            # value streams own the sync ring from t=0) ----
            mf_sb = const.tile([128, 3 + 2 * NB], f32)
            nc.scalar.dma_start(mf_sb[:], miscf)
            w2_sb = const.tile([128, 2, U], f32)
            nc.scalar.dma_start(w2_sb[:], w2)
            w1_sb = const.tile([128, 2, U], vdt)
            nc.scalar.dma_start(w1_sb[:], w1t)
            mb_sb = const.tile([128, 1], bf16)
            nc.scalar.dma_start(mb_sb[:], miscb)

            b1_sb = mf_sb[:, 0:1]
            b2_sb = mf_sb[:, 1:2]
            qT_sb = mf_sb[:, 2:2 + 2 * NB].rearrange("p (k b) -> p k b", k=2)
            ones_sb = mf_sb[:, 2 + 2 * NB:3 + 2 * NB]
            vk_sb = mb_sb[:, 0:1]

            # ones row [1, 128] for the Z broadcast matmul
            onesr_sb = const.tile([1, 128], f32)
            nc.vector.memset(onesr_sb[:], 1.0)

            # ---- h_q^T = relu(W2^T q^T + b2) : [U, NB], once for all batches
            hq_ps = psmall.tile([128, NB], f32, tag="small")
            nc.tensor.matmul(hq_ps[:], w2_sb[:, 0, :], qT_sb[:, 0, :],
                             start=True, stop=False)
            nc.tensor.matmul(hq_ps[:], w2_sb[:, 1, :], qT_sb[:, 1, :],
                             start=False, stop=True)
            hq_sb = const.tile([128, NB], f32)
            nc.scalar.activation(hq_sb[:], hq_ps[:], act.Relu, bias=b2_sb)

            for b in [b for _ in range(repeat) for b in range(NB)]:
                # ---- stream in both layouts of this batch's values (2 DMAs)
                # quarter-interleaved loads (1MB each): compute on quarter q
                # can start as soon as its piece lands; the last-needed piece
                # (vn q3) gates only the final 16 context matmuls.
                vt_sb = vts_pool.tile([128, 2, T], vdt)
                vn_sb = vns_pool.tile([128, NCH, D], bf16)
                vtr = vT[b].rearrange("k p t -> p k t")
                TQ, CQ = T // 4, NCH // 4

                def load_vt(q):
                    nc.sync.dma_start(vt_sb[:, :, ts(q, TQ)],
                                      vtr[:, :, ts(q, TQ)])

                def load_vn(q):
                    nc.sync.dma_start(vn_sb[:, ts(q, CQ), :],
                                      vn[b][:, ts(q, CQ), :])

                # vt pieces front-run vn: the tail pipeline hangs off the
                # last vt piece, while vn's last piece gates only 16 cheap
                # context matmuls.
                load_vt(0); load_vt(1); load_vn(0); load_vt(2)
                load_vn(1); load_vt(3); load_vn(2); load_vn(3)

                e_sb = epool.tile([128, NCH], f32)
                e_bf = epool.tile([128, NCH], bf16)
                cps = cx_pool.tile([1, D], f32)
                WV = 4            # W1 tiles per exp/context wave
                CW = WV * TILE // 128   # score chunks per wave (16)
                for i in range(NT):
                    if i % WV == 0:
                        sc_ps = sc_pool.tile([128, CW], f32, tag="sc")
                    ph = ph_pool.tile([128, TILE], f32)
                    nc.tensor.matmul(ph[:], w1_sb[:, 0, :],
                                     vt_sb[:, 0, ts(i, TILE)],
                                     start=True, stop=False)
                    nc.tensor.matmul(ph[:], w1_sb[:, 1, :],
                                     vt_sb[:, 1, ts(i, TILE)],
                                     start=False, stop=True)
                    # z = relu(h_v + b1) on DVE: (ph + b1) max 0
                    z = zpool.tile([128, TILE], f32)
                    nc.vector.tensor_scalar(z[:], ph[:], b1_sb, 0.0,
                                            op0=alu.add, op1=alu.max)
                    # zt = tanh(z*s + h_q[:, b]); s undoes the x16 fp8 staging
                    zt = ztpool.tile([128, TILE], bf16)
                    nc.scalar.activation(zt[:], z[:], act.Tanh,
                                         bias=hq_sb[:, b:b + 1],
                                         scale=(1.0 / 16.0 if VT_FP8 else 1.0))
                    # score chunks: sc[:, j] = zt_chunk^T @ V
                    for j4 in range(TILE // 128):
                        j = (i % WV) * (TILE // 128) + j4
                        nc.tensor.matmul(sc_ps[:, j:j + 1],
                                         zt[:, ts(j4, 128)], vk_sb,
                                         start=True, stop=True)
                    if i % WV == WV - 1:
                        # wave: exp, bf16 cast, and context matmuls for the
                        # CW chunks just scored -- keeps the batch tail short
                        w = i // WV
                        cs = w * CW
                        nc.scalar.activation(e_sb[:, cs:cs + CW], sc_ps[:],
                                             act.Exp)
                        nc.vector.tensor_copy(e_bf[:, cs:cs + CW],
                                              e_sb[:, cs:cs + CW])
                        for c in range(cs, cs + CW):
                            nc.tensor.matmul(cps[:], e_bf[:, c:c + 1],
                                             vn_sb[:, c, :],
                                             start=(c == 0),
                                             stop=(c == NCH - 1))

                # ---- softmax normalizer
                esum = epool.tile([128, 1], f32)
                nc.vector.reduce_sum(esum[:], e_sb[:],
                                     axis=mybir.AxisListType.X)
                zps = psmall.tile([1, 1], f32, tag="small")
                nc.tensor.matmul(zps[:], esum[:], ones_sb, start=True, stop=True)
                z_sb = epool.tile([1, 1], f32)
                nc.scalar.copy(z_sb[:], zps[:])
                zb_ps = psmall.tile([128, 1], f32, tag="small")
                nc.tensor.matmul(zb_ps[:], onesr_sb[:], z_sb[:],
                                 start=True, stop=True)
                invz = epool.tile([128, 1], f32)
                nc.vector.reciprocal(invz[:], zb_ps[:])

                # ---- attention weights out (no transpose needed)
                attn_sb = outpool.tile([128, NCH], f32)
                nc.vector.tensor_scalar_mul(attn_sb[:], e_sb[:], invz[:])
                nc.scalar.dma_start(attn_r[:, b, :], attn_sb[:])

                # ---- context scale + out
                ctx_sb = outpool.tile([1, D], f32)
                nc.vector.tensor_scalar_mul(ctx_sb[:], cps[:], invz[0:1, :])
                nc.scalar.dma_start(ctx_out[b:b + 1, :], ctx_sb[:])

    nc.compile()
    return nc


def _get_compiled():
    global _compiled
    if _compiled is None:
        _compiled = _build_bass()
    return _compiled


def _stage_inputs(query, values, W1_kernel, W1_bias, W2_kernel, W2_bias,
                  V_kernel, V_bias):
    """Build the 8 per-core input maps (host-side sharding/marshalling)."""
    query = np.asarray(query, dtype=np.float32)
    values = np.asarray(values, dtype=np.float32)
    w1 = np.asarray(W1_kernel, dtype=np.float32)
    w2 = np.asarray(W2_kernel, dtype=np.float32)
    b1 = np.asarray(W1_bias, dtype=np.float32).reshape(U)
    b2 = np.asarray(W2_bias, dtype=np.float32).reshape(U)
    vk = np.asarray(V_kernel, dtype=np.float32).reshape(U)

    if VT_FP8:
        vtype = ml_dtypes.float8_e4m3
        w1s, b1s = w1 * 16.0, b1 * 16.0
    else:
        vtype = BF16
        w1s, b1s = w1, b1
    w1r = np.stack([w1s[0:128, :], w1s[128:256, :]], axis=1).astype(vtype)
    miscb = vk.reshape(128, 1).astype(BF16)
    w2r = np.stack([w2[0:128, :], w2[128:256, :]], axis=1).copy()  # [128, 2, U]

    vals_bf = values.astype(BF16)                           # one bulk cast
    if VT_FP8:
        vals_v = values.astype(vtype)
    else:
        vals_v = vals_bf
    in_maps = []
    for c in range(NCORES):
        sl = slice(c * NB, (c + 1) * NB)
        # vn: row (p, cc) = values[b, p*64 + cc]  -> [NB, 128, NCH, D]
        vnp = vals_bf[sl].reshape(NB, 128, NCH, D)
        # vT: [NB, 2, 128, T] where [b, k, p, i] = values[b, sigma(i), k*128+p]
        # sigma(i) = (i % 128) * 64 + i // 128; equivalently
        # vT[b, k, p, c*128 + q] = values[b, q*64 + c, k*128 + p]
        vt = vals_v[sl].reshape(NB, 128, NCH, 2, 128)       # [b, q, c, k, p]
        vtp = np.ascontiguousarray(vt.transpose(0, 3, 4, 2, 1)  # [b,k,p,c,q]
                                   ).reshape(NB, 2, 128, T)
        qT = query[sl].T                                    # [D, NB]
        miscf = np.zeros((128, 3 + 2 * NB), dtype=np.float32)
        miscf[:, 0] = b1
        miscf[:, 1] = b2
        miscf[:, 2:2 + NB] = qT[0:128, :]
        miscf[:, 2 + NB:2 + 2 * NB] = qT[128:256, :]
        miscf[:, 2 + 2 * NB] = 1.0
        in_maps.append({
            "vT": vtp, "vn": vnp, "miscf": miscf, "w2": w2r, "w1t": w1r,
            "miscb": miscb,
        })
    return in_maps


def _unstage_outputs(results):
    """Gather per-core outputs to full tensors, undoing the T permutation."""
    ctx = np.concatenate([r["ctx_out"] for r in results], axis=0)     # [B, D]
    attn = np.concatenate([r["attn_out"] for r in results], axis=0)   # [B, T]
    return (np.ascontiguousarray(ctx, dtype=np.float32),
            attn.astype(np.float32).reshape(B, T, 1))


def kernel(query, values, W1_kernel, W1_bias, W2_kernel, W2_bias,
           V_kernel, V_bias, _trace=False):
    from concourse import bass_utils

    nc = _get_compiled()
    in_maps = _stage_inputs(query, values, W1_kernel, W1_bias,
                            W2_kernel, W2_bias, V_kernel, V_bias)
    res = bass_utils.run_bass_kernel_spmd(
        nc, in_maps, core_ids=list(range(NCORES)), trace=_trace,
    )
    out = _unstage_outputs(res.results)
    if _trace:
        return out, res
    return out
